# revision 1
# baseline (speedup 1.0000x reference)
"""Single-head causal attention (B=4, L=2048, D=1024) on 8 TRN2 NeuronCores.

Sharding: data-parallel over batch (4) x interleaved query-tile halves (2).
Core 2b+h handles batch b and global q-tiles {h, h+2, ..., h+14} (128 rows
each), so the causal loop-trip counts are identical across cores (SPMD) while
still skipping ~44% of the score/AV work. The kT projection is additionally
split across each core pair (each projects its own 1024-token half) and
assembled with a pair-local AllGather whose ~32 us transfer hides entirely
under the qT/V projections. (V stays duplicated: its 4 MB exchange at the
~62 GB/s collective rate would cost more than the compute it saves.)

The Q/K path (k/q projections, scores) runs fp8e4m3 with DoubleRow matmuls
(256-contraction per instruction): its quantization error is squashed by the
tiny logit scale through softmax. The value path (V projection, E, AV) stays
bf16 because its error reaches the output at full strength. wq/wk are
pre-scaled x256 on host for fp8 range; the 2^16 compensation folds exactly
into the exp scale (2^-21). All PSUM accumulation is f32.

Scores are computed TRANSPOSED: sT[k(128 part), q(free)] so the pad mask (a
per-key quantity) is a per-partition tensor_scalar operand and the softmax
normalizer Z comes from a ones-column appended to V - no partition reductions
or P transposes anywhere. masked_fill is exact:
    cmp[k,q] = (iota_q >= thresh[k]) * padkeep[k]    in {0,1}
    E        = exp(((s' + 960*2^16) * cmp) * 2^-21 - 30)
             = exp(s_raw/32) kept, exp(-30)~=0 masked.
"""
import sys

if "/opt/trn_rl_repo" not in sys.path:
    sys.path.insert(0, "/opt/trn_rl_repo")

import numpy as np
import ml_dtypes

import concourse.bass as bass
import concourse.mybir as mybir
from concourse import bacc, tile
from concourse import bass_utils

F32 = mybir.dt.float32
FP8 = mybir.dt.float8e4
FP8NP = ml_dtypes.float8_e4m3
BF16 = mybir.dt.bfloat16
BF16NP = ml_dtypes.bfloat16

B, L, D = 4, 2048, 1024
NQ = L // 2          # queries per core
NKT = L // 128       # 16 k-tiles
NMC = D // 128       # 8 contraction/model chunks
NQT = NQ // 128      # 8 q-tiles per core
VF = 1025            # v chunk free size (1024 vals + ones col)

SW = 256.0           # host pre-scale for wq, wk (fp8 range)
DR = mybir.MatmulPerfMode.DoubleRow

_NC_CACHE = None


def _build_nc():
    nc = bacc.Bacc(None, target_bir_lowering=False)

    xk_d = nc.dram_tensor("xk", [128, NMC, NQ], FP8, kind="ExternalInput")
    xt16_d = nc.dram_tensor("xt16", [128, NMC, L], BF16, kind="ExternalInput")
    xq_d = nc.dram_tensor("xq", [128, NMC, NQ], FP8, kind="ExternalInput")
    wq_d = nc.dram_tensor("wq", [128, NMC, D], FP8, kind="ExternalInput")
    wk_d = nc.dram_tensor("wk", [128, NMC, D], FP8, kind="ExternalInput")
    wv_d = nc.dram_tensor("wv", [128, NMC, D], BF16, kind="ExternalInput")
    padk_d = nc.dram_tensor("padk", [128, NKT], F32, kind="ExternalInput")
    thr_d = nc.dram_tensor("thr", [128, NKT], F32, kind="ExternalInput")
    out_d = nc.dram_tensor("out", [NQ, D], F32, kind="ExternalOutput")

    AL = mybir.AluOpType
    AF = mybir.ActivationFunctionType

    with tile.TileContext(nc) as tc:
        with (
            tc.tile_pool(name="c", bufs=1) as cpool,
            tc.tile_pool(name="sh", bufs=1) as spool,
            tc.tile_pool(name="wk_", bufs=3) as wpool,
            tc.tile_pool(name="pp", bufs=3, space="PSUM") as pp,
            tc.tile_pool(name="ppo", bufs=2, space="PSUM") as ppo,
            tc.tile_pool(name="ppz", bufs=1, space="PSUM") as ppz,
            tc.tile_pool(name="dr", bufs=1, space="DRAM") as drpool,
        ):
            # persistent tiles; xt16 and E share one slot (disjoint lifetimes)
            xk_sb = cpool.tile([128, NMC, NQ], FP8, name="xk_sb")
            kTo_sb = cpool.tile([128, NMC, NQ], FP8, name="kTo_sb")
            g_in = drpool.tile([128, NMC, NQ], FP8, name="g_in")
            g_out = drpool.tile([2, 128, NMC, NQ], FP8, name="g_out")
            xt16_sb = spool.tile([128, NMC, L], BF16, name="xt16_sb", tag="big")
            wk_sb = cpool.tile([128, NMC, D], FP8, name="wk_sb")
            wv_sb = cpool.tile([128, NMC, D], BF16, name="wv_sb")
            wq_sb = cpool.tile([128, NMC, D], FP8, name="wq_sb")
            xq_sb = cpool.tile([128, NMC, NQ], FP8, name="xq_sb")
            padk_sb = cpool.tile([128, NKT], F32, name="padk_sb")
            thr_sb = cpool.tile([128, NKT], F32, name="thr_sb")
            kT_sb = cpool.tile([128, NMC, L], FP8, name="kT_sb")
            qT_sb = cpool.tile([128, NMC, NQ], FP8, name="qT_sb")
            v_sb = cpool.tile([128, NKT, VF], BF16, name="v_sb")
            iota_sb = cpool.tile([128, NQ], F32, name="iota_sb")
            bias_sb = cpool.tile([128, 1], F32, name="bias_sb")

            # load order: kT-proj inputs first, then qT-proj, then V-proj.
            # wk's first mi-column goes separately so kT's first psum group
            # is gated by ~1.1 MB instead of 2 MB.
            nc.sync.dma_start(wk_sb[:, :, 0:128], wk_d[:, :, 0:128])
            nc.sync.dma_start(xk_sb[:, :, 0:512], xk_d[:, :, 0:512])
            nc.sync.dma_start(xk_sb[:, :, 512:1024], xk_d[:, :, 512:1024])
            nc.sync.dma_start(wk_sb[:, :, 128:1024], wk_d[:, :, 128:1024])
            nc.sync.dma_start(wq_sb[:], wq_d[:])
            nc.sync.dma_start(xq_sb[:], xq_d[:])
            nc.sync.dma_start(padk_sb[:], padk_d[:])
            nc.sync.dma_start(thr_sb[:], thr_d[:])
            nc.sync.dma_start(xt16_sb[:], xt16_d[:])
            nc.sync.dma_start(wv_sb[:], wv_d[:])

            # local q column f (= 128*jl + fi) maps to global q-tile 2*jl + h;
            # iota encodes q_glob - 128*h = 256*jl + fi; thresh data absorbs h.
            nc.gpsimd.iota(
                out=iota_sb[:].rearrange("p (j f) -> p j f", f=128),
                pattern=[[256, NQT], [1, 128]], base=0, channel_multiplier=0,
                allow_small_or_imprecise_dtypes=True,
            )
            nc.vector.memset(bias_sb[:], -30.0)
            nc.vector.memset(v_sb[:, :, D : D + 1], 1.0)

            # PE clock warmup: the HAM gate holds the PE at 1.2 GHz until it
            # sees ~3.4 us of sustained activity. Run junk matmuls on a
            # memset tile during the initial DMA wait (PE is idle anyway) so
            # the real projections start at 2.4 GHz.
            warm_sb = cpool.tile([128, 128], BF16, name="warm_sb")
            nc.vector.memset(warm_sb[:], 0.0)
            ps_w = pp.tile([128, 512], F32, name="ps")
            for wi in range(60):
                nc.tensor.matmul(
                    ps_w[:, 0:128], lhsT=warm_sb[:], rhs=warm_sb[:],
                    start=(wi == 0), stop=(wi == 59),
                )

            # ---- Phase 1a: kT-own[m, tok] = wk.T @ x_own over this core's
            # 1024-token half, then pair-local AllGather to assemble the
            # full kT while qT/V projections keep the PE busy. ----
            for mi in range(NMC):
                ps0 = pp.tile([128, 512], F32, name="ps")
                ps1 = pp.tile([128, 512], F32, name="ps")
                for d in range(0, NMC, 2):
                    for tb, psx in ((0, ps0), (1, ps1)):
                        nc.tensor.matmul(
                            psx[:],
                            lhsT=wk_sb[:, d : d + 2, mi * 128 : (mi + 1) * 128],
                            rhs=xk_sb[:, d : d + 2, tb * 512 : (tb + 1) * 512],
                            start=(d == 0), stop=(d == NMC - 2), perf_mode=DR,
                        )
                for tb, psx in ((0, ps0), (1, ps1)):
                    nc.scalar.copy(kTo_sb[:, mi, tb * 512 : (tb + 1) * 512], psx[:])
            nc.sync.dma_start(g_in[:], kTo_sb[:])
            nc.gpsimd.collective_compute(
                "AllGather", AL.bypass,
                replica_groups=[[0, 1], [2, 3], [4, 5], [6, 7]],
                ins=[g_in[:]], outs=[g_out[:]],
            )
            for r in range(2):
                nc.sync.dma_start(kT_sb[:, :, r * NQ : (r + 1) * NQ], g_out[r])

            # ---- Phase 1c: qT[m, q] = wq.T @ xq ----
            for mi in range(NMC):
                ps0 = pp.tile([128, 512], F32, name="ps")
                ps1 = pp.tile([128, 512], F32, name="ps")
                for d in range(0, NMC, 2):
                    for qb, psx in ((0, ps0), (1, ps1)):
                        nc.tensor.matmul(
                            psx[:],
                            lhsT=wq_sb[:, d : d + 2, mi * 128 : (mi + 1) * 128],
                            rhs=xq_sb[:, d : d + 2, qb * 512 : (qb + 1) * 512],
                            start=(d == 0), stop=(d == NMC - 2), perf_mode=DR,
                        )
                for qb, psx in ((0, ps0), (1, ps1)):
                    nc.scalar.copy(qT_sb[:, mi, qb * 512 : (qb + 1) * 512], psx[:])

            # ---- Phase 1b: V[tok, m] = x @ wv in bf16 (value path stays
            # high precision: its quantization error hits the output at
            # full strength, unlike the Q/K path) ----
            for kt in range(NKT):
                ps0 = pp.tile([128, 512], F32, name="ps")
                ps1 = pp.tile([128, 512], F32, name="ps")
                for d in range(NMC):
                    for mb, psx in ((0, ps0), (1, ps1)):
                        nc.tensor.matmul(
                            psx[:],
                            lhsT=xt16_sb[:, d, kt * 128 : (kt + 1) * 128],
                            rhs=wv_sb[:, d, mb * 512 : (mb + 1) * 512],
                            start=(d == 0), stop=(d == NMC - 1),
                        )
                for mb, psx in ((0, ps0), (1, ps1)):
                    nc.scalar.copy(v_sb[:, kt, mb * 512 : (mb + 1) * 512], psx[:])

            # ---- Phase 2: scores (transposed) + mask + exp, per k-tile ----
            # Local q-tile jl holds global q-tile 2*jl + h, so k-tile kt is
            # causally live only for jl >= kt//2: a contiguous tail of the
            # local q axis. Fully-dead (kt, jl) pairs are skipped; the h=0
            # core's extra tile per jl is killed by cmp data.
            E_sb = spool.tile([128, NKT, NQ], BF16, name="E_sb", tag="big")
            for kt in range(NKT):
                jl0 = kt // 2
                f0 = jl0 * 128
                cmp = wpool.tile([128, NQ], F32, name="cmp", bufs=2)
                nc.vector.tensor_scalar(
                    out=cmp[:, f0:], in0=iota_sb[:, f0:],
                    scalar1=thr_sb[:, kt : kt + 1], scalar2=padk_sb[:, kt : kt + 1],
                    op0=AL.is_ge, op1=AL.mult,
                )
                s_sb = wpool.tile([128, NQ], F32, name="s_sb", bufs=3)
                f = f0
                while f < NQ:
                    w = min(512, NQ - f)
                    ps = pp.tile([128, 512], F32, name="ps")
                    for m in range(0, NMC, 2):
                        nc.tensor.matmul(
                            ps[:, 0:w],
                            lhsT=kT_sb[:, m : m + 2, kt * 128 : (kt + 1) * 128],
                            rhs=qT_sb[:, m : m + 2, f : f + w],
                            start=(m == 0), stop=(m == NMC - 2), perf_mode=DR,
                        )
                    nc.vector.scalar_tensor_tensor(
                        out=s_sb[:, f : f + w], in0=ps[:, 0:w],
                        scalar=62914560.0,  # 960 * 2^16
                        in1=cmp[:, f : f + w],
                        op0=AL.add, op1=AL.mult,
                    )
                    f += w
                nc.scalar.activation(
                    out=E_sb[:, kt, f0:], in_=s_sb[:, f0:],
                    func=AF.Exp, bias=bias_sb[:], scale=2.0 ** -21,
                )

                # ---- Phase 3 (interleaved): after scores for kt = 2*jl+1,
                # q-tile jl has all its E tiles -> emit its AV + normalize.
                # out[q,m] = (E^T @ [V|1])[q,m] / Z[q]
                if kt % 2 == 1:
                    jl = (kt - 1) // 2
                    nkt = 2 * jl + 2  # causally-live k-tiles for this q-tile
                    po = ppo.tile([128, D], F32, name="po")
                    pz = ppz.tile([128, 1], F32, name="pz")
                    for kta in range(nkt):
                        lhsT = E_sb[:, kta, jl * 128 : (jl + 1) * 128]
                        nc.tensor.matmul(po[:, 0:512], lhsT=lhsT,
                                         rhs=v_sb[:, kta, 0:512],
                                         start=(kta == 0), stop=(kta == nkt - 1))
                        nc.tensor.matmul(po[:, 512:1024], lhsT=lhsT,
                                         rhs=v_sb[:, kta, 512:1024],
                                         start=(kta == 0), stop=(kta == nkt - 1))
                        nc.tensor.matmul(pz[:], lhsT=lhsT,
                                         rhs=v_sb[:, kta, D : D + 1],
                                         start=(kta == 0), stop=(kta == nkt - 1))
                    rec = wpool.tile([128, 1], F32, name="rec", bufs=2)
                    nc.vector.reciprocal(rec[:], pz[:])
                    o_sb = wpool.tile([128, D], F32, name="o_sb", bufs=3)
                    # halves: the first store overlaps the second normalize
                    for ob in range(2):
                        sl = slice(ob * 512, (ob + 1) * 512)
                        nc.vector.tensor_scalar(
                            out=o_sb[:, sl], in0=po[:, sl], scalar1=rec[:],
                            scalar2=None, op0=AL.mult,
                        )
                        nc.sync.dma_start(out_d[jl * 128 : (jl + 1) * 128, sl],
                                          o_sb[:, sl])

    nc.compile()
    return nc


def _chunked(a):
    """[C*128, N] -> [128, C, N] contiguous."""
    c = a.shape[0] // 128
    return np.ascontiguousarray(a.reshape(c, 128, *a.shape[1:]).transpose(1, 0, 2))


def _qsel(h):
    """Global query rows handled by half h: interleaved 128-row q-tiles."""
    return np.concatenate(
        [np.arange(128 * (2 * jl + h), 128 * (2 * jl + h) + 128) for jl in range(NQT)]
    )


def build_in_maps(inputs):
    x = np.asarray(inputs["x"], dtype=np.float32)
    pad = np.asarray(inputs["pad_mask"])
    wq_h = _chunked(np.asarray(inputs["wq"], dtype=np.float32) * SW).astype(FP8NP)
    wk_h = _chunked(np.asarray(inputs["wk"], dtype=np.float32) * SW).astype(FP8NP)
    wv_h = _chunked(np.asarray(inputs["wv"], dtype=np.float32)).astype(BF16NP)

    in_maps = []
    for c in range(8):
        b, h = divmod(c, 2)
        qsel = _qsel(h)
        xtb16 = _chunked(x[b].T).astype(BF16NP)            # [128, 8, 2048]
        xkb = _chunked(x[b, h * NQ : (h + 1) * NQ, :].T).astype(FP8NP)
        xqb = _chunked(x[b, qsel, :].T).astype(FP8NP)      # [128, 8, 1024]
        keep = (~pad[b]).astype(np.float32)                     # [2048]
        padk = np.ascontiguousarray(keep.reshape(NKT, 128).T)   # [128, 16]
        # keep iff iota (= q_glob - 128h) >= thresh = 128*kt + p - 128*h
        thr = (
            np.add.outer(np.arange(128, dtype=np.float32),
                         128.0 * np.arange(NKT, dtype=np.float32))
            - np.float32(128 * h)
        ).astype(np.float32)                                    # [128, 16]
        in_maps.append({
            "xk": xkb, "xt16": xtb16, "xq": xqb, "wq": wq_h, "wk": wk_h,
            "wv": wv_h, "padk": padk, "thr": np.ascontiguousarray(thr),
        })
    return in_maps


def kernel(**inputs):
    global _NC_CACHE
    if _NC_CACHE is None:
        _NC_CACHE = _build_nc()
    nc = _NC_CACHE

    in_maps = build_in_maps(inputs)
    res = bass_utils.run_bass_kernel_spmd(nc, in_maps, core_ids=list(range(8)))
    out = np.empty((B, L, D), dtype=np.float32)
    for b in range(B):
        for h in range(2):
            out[b, _qsel(h)] = res.results[2 * b + h]["out"]
    return out



# revision 2
# speedup vs baseline: 1.5435x; 1.5435x over previous
"""Single-head causal attention (B=4, L=2048, D=1024) on 8 TRN2 NeuronCores.

Sharding: data-parallel over batch (4) x interleaved query-tile halves (2).
Core 2b+h handles batch b and global q-tiles {h, h+2, ..., h+14}.

Padded keys (~half of all keys) are compressed out on the host: the kernel
only projects/scores the kept keys (capacity CT*128 slots, CT derived from
the actual inputs at compile time with a recompile guard). Masked logits map
to E = exp(-512) = exact 0, so skipped/dummy slots contribute nothing. A
virtual key in slot 0 (kT column = 0, value row = mean of ALL value rows,
cmp scale 0.94140625 -> E = exp(-30) for every query) reproduces the
reference's fully-masked-row semantics (uniform average over all 2048 keys)
exactly while perturbing normal rows by ~1e-13.

Work split within a core pair: the kT projection is split by token halves
and the V projection by column halves; each is assembled with a pair-local
AllGather (~193 GB/s) that hides under the projection/scores phases. The
Q/K path runs fp8e4m3 with DoubleRow matmuls; V/E/AV stay bf16. wq/wk are
pre-scaled x256 on host; the 2^16 compensation folds into the exp scale
(2^-21). Scores are computed TRANSPOSED: sT[k(128 part), q(free)] so the
mask is a per-partition tensor_scalar operand and the softmax normalizer Z
comes from a ones-column matmul - no partition reductions anywhere.
masked_fill:
    cmp[k,q] = (iota_q >= thresh[k]) * kscale[k]
    E        = exp(((s' + 2^30) * cmp) * 2^-21 - 512)
             = exp(s_raw/32) kept, 0 masked (exp(-512) underflows),
               exp(-30) virtual.

Static causal pruning is data-specialized: scores for k-tile kt start at
q-tile jl0[kt]; AV for q-tile jl accumulates nkt[jl] k-tiles; both derived
from the compressed key positions (min over batches/halves, so the shared
SPMD program covers every core; per-core dead regions fall out as E = 0).
"""
import sys

if "/opt/trn_rl_repo" not in sys.path:
    sys.path.insert(0, "/opt/trn_rl_repo")

import numpy as np
import ml_dtypes

import concourse.bass as bass
import concourse.mybir as mybir
from concourse import bacc, tile
from concourse import bass_utils

F32 = mybir.dt.float32
FP8 = mybir.dt.float8e4
FP8NP = ml_dtypes.float8_e4m3
BF16 = mybir.dt.bfloat16
BF16NP = ml_dtypes.bfloat16

B, L, D = 4, 2048, 1024
NQ = L // 2          # queries per core
NMC = D // 128       # 8 contraction/model chunks
NQT = NQ // 128      # 8 q-tiles per core

SW = 256.0           # host pre-scale for wq, wk (fp8 range)
C0 = float(2 ** 30)  # additive pre-mask constant; C0 * 2^-21 = 512
GAMMA = 1.0 - 30.0 / 512.0  # virtual-key cmp scale -> E = exp(-30)
DR = mybir.MatmulPerfMode.DoubleRow

_NC_CACHE = None
_SPEC_CACHE = None


def _make_spec(pad_mask):
    """Static program parameters derived from the pad mask."""
    pad = np.asarray(pad_mask)
    kept = [np.flatnonzero(~pad[b]) for b in range(B)]
    maxk = max(len(k) for k in kept) + 1          # +1 virtual slot
    ct = (maxk + 127) // 128
    nc_keys = ct * 128
    # min over batches of the original position of each tile's first slot
    # (virtual = -inf, dummies = +inf)
    minpos = []
    for kt in range(ct):
        m = np.inf
        for b in range(B):
            slot = kt * 128
            if slot == 0:
                m = -np.inf
            elif slot <= len(kept[b]):
                m = min(m, float(kept[b][slot - 1]))
        minpos.append(m)
    jl0 = []
    for kt in range(ct):
        first = NQT
        for h in (0, 1):
            for jl in range(NQT):
                if 128 * (2 * jl + h) + 127 >= minpos[kt]:
                    first = min(first, jl)
                    break
    # a tile whose first slot is a dummy on every batch is fully dead
        jl0.append(first)
    nkt = []
    for jl in range(NQT):
        last = 0
        for kt in range(ct):
            if minpos[kt] <= 128 * (2 * jl + 1) + 127:
                last = kt
        nkt.append(last + 1)
    return (ct, nc_keys // 2, tuple(jl0), tuple(nkt))


def _build_nc(spec):
    ct, ht, jl0, nkt = spec
    nck = ct * 128       # compressed key slots
    nc = bacc.Bacc(None, target_bir_lowering=False)

    xk_d = nc.dram_tensor("xk", [128, NMC, ht], FP8, kind="ExternalInput")
    xt16_d = nc.dram_tensor("xt16", [128, NMC, nck], BF16, kind="ExternalInput")
    xq_d = nc.dram_tensor("xq", [128, NMC, NQ], FP8, kind="ExternalInput")
    wq_d = nc.dram_tensor("wq", [128, NMC, D], FP8, kind="ExternalInput")
    wk_d = nc.dram_tensor("wk", [128, NMC, D], FP8, kind="ExternalInput")
    wv_d = nc.dram_tensor("wv", [128, NMC, 512], BF16, kind="ExternalInput")
    ksc_d = nc.dram_tensor("ksc", [128, ct], F32, kind="ExternalInput")
    thr_d = nc.dram_tensor("thr", [128, ct], F32, kind="ExternalInput")
    out_d = nc.dram_tensor("out", [NQ, D], F32, kind="ExternalOutput")

    AL = mybir.AluOpType
    AF = mybir.ActivationFunctionType

    with tile.TileContext(nc) as tc:
        with (
            tc.tile_pool(name="c", bufs=1) as cpool,
            tc.tile_pool(name="sh", bufs=1) as spool,
            tc.tile_pool(name="wk_", bufs=3) as wpool,
            tc.tile_pool(name="pp", bufs=3, space="PSUM") as pp,
            tc.tile_pool(name="ppo", bufs=2, space="PSUM") as ppo,
            tc.tile_pool(name="ppz", bufs=1, space="PSUM") as ppz,
            tc.tile_pool(name="dr", bufs=1, space="DRAM") as drpool,
        ):
            # persistent tiles; xt16 and E share one slot (disjoint lifetimes)
            xk_sb = cpool.tile([128, NMC, ht], FP8, name="xk_sb")
            kTo_sb = cpool.tile([128, NMC, ht], FP8, name="kTo_sb")
            vo_sb = cpool.tile([128, ct, 512], BF16, name="vo_sb")
            g_in = drpool.tile([128, NMC, ht], FP8, name="g_in")
            g_out = drpool.tile([2, 128, NMC, ht], FP8, name="g_out")
            g2_in = drpool.tile([128, ct, 512], BF16, name="g2_in")
            g2_out = drpool.tile([2, 128, ct, 512], BF16, name="g2_out")
            xt16_sb = spool.tile([128, NMC, nck], BF16, name="xt16_sb", tag="big")
            wk_sb = cpool.tile([128, NMC, D], FP8, name="wk_sb")
            wv_sb = cpool.tile([128, NMC, 512], BF16, name="wv_sb")
            wq_sb = cpool.tile([128, NMC, D], FP8, name="wq_sb")
            xq_sb = cpool.tile([128, NMC, NQ], FP8, name="xq_sb")
            ksc_sb = cpool.tile([128, ct], F32, name="ksc_sb")
            thr_sb = cpool.tile([128, ct], F32, name="thr_sb")
            kT_sb = cpool.tile([128, NMC, nck], FP8, name="kT_sb")
            qT_sb = cpool.tile([128, NMC, NQ], FP8, name="qT_sb")
            v_sb = cpool.tile([128, ct, 1025], BF16, name="v_sb")
            iota_sb = cpool.tile([128, NQ], F32, name="iota_sb")
            bias_sb = cpool.tile([128, 1], F32, name="bias_sb")

            # load order: kT-proj inputs first, then V-proj, then qT-proj.
            nc.sync.dma_start(wk_sb[:, :, 0:128], wk_d[:, :, 0:128])
            nc.sync.dma_start(xk_sb[:], xk_d[:])
            nc.sync.dma_start(wk_sb[:, :, 128:1024], wk_d[:, :, 128:1024])
            nc.sync.dma_start(xt16_sb[:, :, 0:nck//2], xt16_d[:, :, 0:nck//2])
            nc.sync.dma_start(xt16_sb[:, :, nck//2:], xt16_d[:, :, nck//2:])
            nc.sync.dma_start(wv_sb[:], wv_d[:])
            nc.sync.dma_start(wq_sb[:], wq_d[:])
            nc.sync.dma_start(xq_sb[:], xq_d[:])
            nc.sync.dma_start(ksc_sb[:], ksc_d[:])
            nc.sync.dma_start(thr_sb[:], thr_d[:])

            # local q column f (= 128*jl + fi) maps to global q-tile 2*jl + h;
            # iota encodes q_glob - 128*h = 256*jl + fi; thresh data absorbs h.
            nc.gpsimd.iota(
                out=iota_sb[:].rearrange("p (j f) -> p j f", f=128),
                pattern=[[256, NQT], [1, 128]], base=0, channel_multiplier=0,
                allow_small_or_imprecise_dtypes=True,
            )
            nc.vector.memset(bias_sb[:], -512.0)
            nc.vector.memset(v_sb[:, :, 1024:1025], 1.0)

            # PE clock warmup: the HAM gate holds the PE at low clock until it
            # sees a few us of sustained activity. Run junk matmuls on a
            # memset tile during the initial DMA wait (PE is idle anyway) so
            # the real projections start at full clock.
            warm_sb = cpool.tile([128, 128], BF16, name="warm_sb")
            nc.vector.memset(warm_sb[:], 0.0)
            ps_w = pp.tile([128, 512], F32, name="ps")
            for wi in range(40):
                nc.tensor.matmul(
                    ps_w[:, 0:128], lhsT=warm_sb[:], rhs=warm_sb[:],
                    start=(wi == 0), stop=(wi == 39),
                )

            # ---- Phase 1a: kT-own[m, tok] = wk.T @ x_own over this core's
            # ht-token half of the compressed keys, then pair-local AllGather
            # to assemble the full kT while V/qT projections keep the PE
            # busy. g_in is staged per-mi so the AG trigger fires right after
            # the last chunk evacuates. ----
            for mi in range(NMC):
                f = 0
                while f < ht:
                    w = min(512, ht - f)
                    ps = pp.tile([128, 512], F32, name="ps")
                    for d in range(0, NMC, 2):
                        nc.tensor.matmul(
                            ps[:, 0:w],
                            lhsT=wk_sb[:, d : d + 2, mi * 128 : (mi + 1) * 128],
                            rhs=xk_sb[:, d : d + 2, f : f + w],
                            start=(d == 0), stop=(d == NMC - 2), perf_mode=DR,
                        )
                    nc.scalar.copy(kTo_sb[:, mi, f : f + w], ps[:, 0:w])
                    f += w
                nc.sync.dma_start(g_in[:, mi], kTo_sb[:, mi])
            nc.gpsimd.collective_compute(
                "AllGather", AL.bypass,
                replica_groups=[[0, 1], [2, 3], [4, 5], [6, 7]],
                ins=[g_in[:]], outs=[g_out[:]],
            )
            for r in range(2):
                nc.sync.dma_start(kT_sb[:, :, r * ht : (r + 1) * ht], g_out[r])

            # ---- Phase 1b: V-own[tok, mo] = x_c @ wv_own in bf16 over this
            # core's 512-column half (value path stays high precision), then
            # pair-local AllGather assembles full V. Rank order = global
            # column order, so v_sb is identical on both cores. ----
            for kt in range(ct):
                ps = pp.tile([128, 512], F32, name="ps")
                for d in range(NMC):
                    nc.tensor.matmul(
                        ps[:],
                        lhsT=xt16_sb[:, d, kt * 128 : (kt + 1) * 128],
                        rhs=wv_sb[:, d],
                        start=(d == 0), stop=(d == NMC - 1),
                    )
                nc.scalar.copy(vo_sb[:, kt], ps[:])
                nc.sync.dma_start(g2_in[:, kt], vo_sb[:, kt])
            nc.gpsimd.collective_compute(
                "AllGather", AL.bypass,
                replica_groups=[[0, 1], [2, 3], [4, 5], [6, 7]],
                ins=[g2_in[:]], outs=[g2_out[:]],
            )
            for r in range(2):
                nc.sync.dma_start(v_sb[:, :, r * 512 : (r + 1) * 512], g2_out[r])

            # ---- Phase 1c: qT[m, q] = wq.T @ xq ----
            for mi in range(NMC):
                ps0 = pp.tile([128, 512], F32, name="ps")
                ps1 = pp.tile([128, 512], F32, name="ps")
                for d in range(0, NMC, 2):
                    for qb, psx in ((0, ps0), (1, ps1)):
                        nc.tensor.matmul(
                            psx[:],
                            lhsT=wq_sb[:, d : d + 2, mi * 128 : (mi + 1) * 128],
                            rhs=xq_sb[:, d : d + 2, qb * 512 : (qb + 1) * 512],
                            start=(d == 0), stop=(d == NMC - 2), perf_mode=DR,
                        )
                for qb, psx in ((0, ps0), (1, ps1)):
                    nc.scalar.copy(qT_sb[:, mi, qb * 512 : (qb + 1) * 512], psx[:])

            # ---- Phase 2: scores (transposed) + mask + exp, per k-tile ----
            E_sb = spool.tile([128, ct, NQ], BF16, name="E_sb", tag="big")
            for kt in range(ct):
                if jl0[kt] >= NQT:
                    continue
                f0 = jl0[kt] * 128
                cmp = wpool.tile([128, NQ], F32, name="cmp", bufs=2)
                nc.vector.tensor_scalar(
                    out=cmp[:, f0:], in0=iota_sb[:, f0:],
                    scalar1=thr_sb[:, kt : kt + 1], scalar2=ksc_sb[:, kt : kt + 1],
                    op0=AL.is_ge, op1=AL.mult,
                )
                s_sb = wpool.tile([128, NQ], F32, name="s_sb", bufs=3)
                f = f0
                while f < NQ:
                    w = min(512, NQ - f)
                    ps = pp.tile([128, 512], F32, name="ps")
                    for m in range(0, NMC, 2):
                        nc.tensor.matmul(
                            ps[:, 0:w],
                            lhsT=kT_sb[:, m : m + 2, kt * 128 : (kt + 1) * 128],
                            rhs=qT_sb[:, m : m + 2, f : f + w],
                            start=(m == 0), stop=(m == NMC - 2), perf_mode=DR,
                        )
                    nc.vector.scalar_tensor_tensor(
                        out=s_sb[:, f : f + w], in0=ps[:, 0:w],
                        scalar=C0,
                        in1=cmp[:, f : f + w],
                        op0=AL.add, op1=AL.mult,
                    )
                    f += w
                nc.scalar.activation(
                    out=E_sb[:, kt, f0:], in_=s_sb[:, f0:],
                    func=AF.Exp, bias=bias_sb[:], scale=2.0 ** -21,
                )

            # ---- Phase 3: AV + normalize per q-tile.
            # out[q,m] = (E^T @ [V|1])[q,m] / Z[q]
            for jl in range(NQT):
                n = nkt[jl]
                po = ppo.tile([128, D], F32, name="po")
                pz = ppz.tile([128, 1], F32, name="pz")
                for kta in range(n):
                    lhsT = E_sb[:, kta, jl * 128 : (jl + 1) * 128]
                    nc.tensor.matmul(po[:, 0:512], lhsT=lhsT,
                                     rhs=v_sb[:, kta, 0:512],
                                     start=(kta == 0), stop=(kta == n - 1))
                    nc.tensor.matmul(po[:, 512:1024], lhsT=lhsT,
                                     rhs=v_sb[:, kta, 512:1024],
                                     start=(kta == 0), stop=(kta == n - 1))
                    nc.tensor.matmul(pz[:], lhsT=lhsT,
                                     rhs=v_sb[:, kta, 1024:1025],
                                     start=(kta == 0), stop=(kta == n - 1))
                rec = wpool.tile([128, 1], F32, name="rec", bufs=2)
                nc.vector.reciprocal(rec[:], pz[:])
                o_sb = wpool.tile([128, D], F32, name="o_sb", bufs=3)
                # halves: the first store overlaps the second normalize
                for ob in range(2):
                    sl = slice(ob * 512, (ob + 1) * 512)
                    nc.vector.tensor_scalar(
                        out=o_sb[:, sl], in0=po[:, sl], scalar1=rec[:],
                        scalar2=None, op0=AL.mult,
                    )
                    nc.sync.dma_start(out_d[jl * 128 : (jl + 1) * 128, sl],
                                      o_sb[:, sl])

    nc.compile()
    return nc


def _chunked(a):
    """[C*128, N] -> [128, C, N] contiguous."""
    c = a.shape[0] // 128
    return np.ascontiguousarray(a.reshape(c, 128, *a.shape[1:]).transpose(1, 0, 2))


def _qsel(h):
    """Global query rows handled by half h: interleaved 128-row q-tiles."""
    return np.concatenate(
        [np.arange(128 * (2 * jl + h), 128 * (2 * jl + h) + 128) for jl in range(NQT)]
    )


def build_in_maps(inputs, spec=None):
    x = np.asarray(inputs["x"], dtype=np.float32)
    pad = np.asarray(inputs["pad_mask"])
    if spec is None:
        spec = _make_spec(pad)
    ct, ht, _, _ = spec
    nck = ct * 128
    wq_h = _chunked(np.asarray(inputs["wq"], dtype=np.float32) * SW).astype(FP8NP)
    wk_h = _chunked(np.asarray(inputs["wk"], dtype=np.float32) * SW).astype(FP8NP)
    wv_f = np.asarray(inputs["wv"], dtype=np.float32)

    in_maps = []
    for c in range(8):
        b, h = divmod(c, 2)
        kept = np.flatnonzero(~pad[b])
        nk = len(kept)
        # compressed x: slot 0 = virtual key (kT col 0, value row = mean x)
        xc = np.zeros((nck, D), np.float32)
        xc[1 : 1 + nk] = x[b, kept]
        xcv = xc.copy()
        xcv[0] = x[b].mean(axis=0)
        thr = np.full(nck, 1e9, np.float32)
        thr[0] = -1e9
        thr[1 : 1 + nk] = kept.astype(np.float32) - 128.0 * h
        ksc = np.ones(nck, np.float32)
        ksc[0] = GAMMA

        qsel = _qsel(h)
        xkb = _chunked(xc[h * ht : (h + 1) * ht].T).astype(FP8NP)
        xtb16 = _chunked(xcv.T).astype(BF16NP)               # [128, 8, nck]
        xqb = _chunked(x[b, qsel, :].T).astype(FP8NP)        # [128, 8, 1024]
        wvb = _chunked(wv_f[:, h * 512 : (h + 1) * 512]).astype(BF16NP)
        in_maps.append({
            "xk": xkb, "xt16": xtb16, "xq": xqb, "wq": wq_h, "wk": wk_h,
            "wv": wvb,
            "ksc": np.ascontiguousarray(ksc.reshape(ct, 128).T),
            "thr": np.ascontiguousarray(thr.reshape(ct, 128).T),
        })
    return in_maps


def _ensure_compiled(inputs):
    global _NC_CACHE, _SPEC_CACHE
    spec = _make_spec(np.asarray(inputs["pad_mask"]))
    if _NC_CACHE is None or _SPEC_CACHE != spec:
        _NC_CACHE = _build_nc(spec)
        _SPEC_CACHE = spec
    return _NC_CACHE, spec


def kernel(**inputs):
    nc, spec = _ensure_compiled(inputs)
    in_maps = build_in_maps(inputs, spec)
    res = bass_utils.run_bass_kernel_spmd(nc, in_maps, core_ids=list(range(8)))
    out = np.empty((B, L, D), dtype=np.float32)
    for b in range(B):
        for h in range(2):
            out[b, _qsel(h)] = res.results[2 * b + h]["out"]
    return out
